# revision 15
# baseline (speedup 1.0000x reference)
"""Trainium2 Bass kernel for nn_LorentzGraphConvolution (v4).

Row-sharded across 8 NeuronCores: core c owns rows [c*1536, (c+1)*1536) of
the attention matrix / output. Every core redundantly computes the tiny
linear phase (h, k for all N; q for its local rows) from broadcast inputs,
so no collectives are needed.

Pipeline design (engine budget per core, phase C):
  - ACT (the wall): one Sigmoid ACTIVATE per j-TRIPLE [128,1536] from PSUM
    (~1.5us per 3 tiles). Whole kernel uses one act-table set
    (sigmoid_and_others: sigmoid+square+copy) -> no table reloads.
  - PE: row-packed MM1 pairs (K=64+64 concurrent) and col-tiled MM2 pairs
    (M=64 to PSUM partitions 0:64 / 64:128 concurrent). No mask matmuls.
  - DVE: applies the adjacency mask as one multiply per triple
    (sig * adjT, bf16) between sigmoid and MM2.
  - DMA: adjT is pre-transposed and pre-cast to bf16 on the HOST (free),
    streamed in 6-tile "hex" groups with 6KB contiguous per-partition
    lines.
  - PSUM: the linear phase's pool is scoped and released before phase C,
    freeing banks for double-buffered [128,1536] attention triples
    (3+3) + supT (1) + transpose scratch (1) = 8 banks.
"""

import math
import os
import sys
from contextlib import ExitStack

for _p in ("/opt/trn_rl_repo", "/root/.axon_site/_ro/trn_rl_repo", "/root/.axon_site"):
    if os.path.isdir(_p) and _p not in sys.path:
        sys.path.insert(0, _p)

import ml_dtypes
import numpy as np

import concourse.bass as bass
import concourse.tile as tile
from concourse import bacc, bass_utils, masks, mybir
from concourse.tile import add_dep_helper

DT = mybir.dt
F32 = DT.float32
BF16 = DT.bfloat16
AF = mybir.ActivationFunctionType
ALU = mybir.AluOpType

N_FULL = 12288
D = 64
N_CORES = 8
R_FULL = N_FULL // N_CORES  # 1536 rows per core


def emit(tc, io, nn, rr, esc, esc_q, esc_k, sig_scale, sig_bias):
    nc = tc.nc
    TJ = nn // 128          # 96 global j tiles
    TL = rr // 128          # 12 local i tiles
    NCH = 3                 # i-chunks per core
    IC = rr // NCH          # 512 rows per chunk
    NHEX = TJ // 6          # 16 hex groups of j tiles per chunk
    assert IC == 512 and TJ % 6 == 0

    ctx = ExitStack()

    const = ctx.enter_context(tc.tile_pool(name="const", bufs=1))
    persist = ctx.enter_context(tc.tile_pool(name="persist", bufs=1))
    slab = ctx.enter_context(tc.tile_pool(name="slab", bufs=1))
    flat = ctx.enter_context(tc.tile_pool(name="flat", bufs=2))
    oneshot = ctx.enter_context(tc.tile_pool(name="oneshot", bufs=1))
    wide = ctx.enter_context(tc.tile_pool(name="wide", bufs=2))
    small = ctx.enter_context(tc.tile_pool(name="small", bufs=4))
    hex_pool = ctx.enter_context(tc.tile_pool(name="hexs", bufs=3))
    sig_pool = ctx.enter_context(tc.tile_pool(name="sig", bufs=3))
    sigm_pool = ctx.enter_context(tc.tile_pool(name="sigm", bufs=3))
    out_pool = ctx.enter_context(tc.tile_pool(name="outp", bufs=4))

    # ---- constants / small inputs -------------------------------------
    xT_s = flat.tile([65, nn], BF16, tag="flat")
    nc.sync.dma_start(xT_s[:], io["xT"][:])
    xqT_s = const.tile([65, rr], BF16)
    nc.sync.dma_start(xqT_s[:], io["xqT"][:])
    wT_s = const.tile([65, 64], BF16)
    nc.sync.dma_start(wT_s[:], io["wT"][:])
    wqT_s = const.tile([65, 64], BF16)
    nc.sync.dma_start(wqT_s[:], io["wqT"][:])
    wkT_s = const.tile([65, 64], BF16)
    nc.sync.dma_start(wkT_s[:], io["wkT"][:])
    ident = const.tile([64, 64], F32)
    masks.make_identity(nc, ident[:])
    sig_bias_ap = const.tile([128, 1], F32)
    nc.vector.memset(sig_bias_ap[:], sig_bias)
    I32 = DT.int32
    magic = const.tile([128, 1], I32)
    nc.vector.memset(magic[:], 0x5F3759DF)

    def fast_rsqrt(dst, x, tmp_pool, nb, tag):
        """dst = 1/sqrt(x) via bit-trick + 2 Newton iterations (DVE only;
        keeps Sqrt off the ACT engine so phase C never swaps act tables)."""
        xi = x.bitcast(I32)
        sh = tmp_pool.tile([128, nb], I32, tag=tag + "sh", name="sh", bufs=2)
        nc.vector.tensor_scalar(sh[:], xi, 1, None, ALU.arith_shift_right)
        y = dst
        nc.vector.tensor_tensor(y.bitcast(I32), magic[:].to_broadcast((128, nb)),
                                sh[:], ALU.subtract)
        for _ in range(2):
            ysq = tmp_pool.tile([128, nb], F32, tag=tag + "ysq", name="ysq",
                                bufs=2)
            nc.vector.tensor_tensor(ysq[:], y, y, ALU.mult)
            t = tmp_pool.tile([128, nb], F32, tag=tag + "t", name="t", bufs=2)
            nc.vector.tensor_tensor(t[:], ysq[:], x, ALU.mult)
            w = tmp_pool.tile([128, nb], F32, tag=tag + "w", name="w", bufs=2)
            nc.vector.tensor_scalar(w[:], t[:], -0.5, 1.5, ALU.mult, ALU.add)
            yn = tmp_pool.tile([128, nb], F32, tag=tag + "yn", name="yn",
                               bufs=2)
            nc.vector.tensor_tensor(yn[:], y, w[:], ALU.mult)
            y = yn[:]
        nc.vector.tensor_copy(dst, y)

    # persistent per-core tensors. "pad" slabs put tile t's 64 features in
    # cols [t*128, t*128+64) so a 128x128 block DMA-transpose lands the
    # features at partitions 0:64; col 64 holds the bias-ones row.
    hpad = persist.tile([128, TJ * 128], BF16)
    hpad3 = hpad.rearrange("p (t c) -> p t c", c=128)
    # k^T stacked pairs: block t' rows 0:64 = kT[2t'], rows 64:128 = kT[2t'+1]
    kT_stk = persist.tile([128, (TJ // 2) * 128], BF16)
    # qm^T duplicated in both partition halves for the row-packed MM1 pairs
    qmT_full = persist.tile([128, TL * 128], BF16)

    # =========== linear phase (scoped PSUM pool) ======================
    with tc.tile_pool(name="psL", bufs=2, space="PSUM") as psL:

        def linear_array(T, lhsT_fn, rhs_w, esc_, neg, dest3, pref):
            """dest3: [128, T, c] bf16 view; writes normalized rows:
            col 0 = time, cols 1:64 = scaled spatial."""
            tot = slab.tile([128, T], F32, tag=pref + "tot", name=pref + "tot")
            logit = slab.tile([128, T], F32, tag=pref + "lg", name=pref + "lg")
            for b0 in range(0, T, 8):
                nb = min(8, T - b0)
                ps = psL.tile([128, 512], F32, tag="linps", name="linps")
                ps = ps[:, : nb * 64]
                ps3 = ps.rearrange("p (t d) -> p t d", d=64)
                for u in range(nb):
                    nc.tensor.matmul(ps[:, u * 64:(u + 1) * 64],
                                     lhsT_fn(b0 + u), rhs_w, start=True,
                                     stop=True)
                sqf = wide.tile([128, 512], F32, tag="sqw", name="sqw")
                sqf = sqf[:, : nb * 64]
                nc.scalar.activation(sqf, ps, AF.Square)
                sqf3 = sqf.rearrange("p (t d) -> p t d", d=64)
                nc.vector.tensor_reduce(tot[:, b0:b0 + nb], sqf3,
                                        axis=mybir.AxisListType.X, op=ALU.add)
                nc.vector.tensor_copy(logit[:, b0:b0 + nb], ps3[:, :, 0])
                # psum->slab copy on ACT (Copy shares the sigmoid table set)
                nc.scalar.activation(dest3[:, b0:b0 + nb, 0:64], ps3, AF.Copy)
            # slab-level Lorentz normalization
            sg = slab.tile([128, T], F32, tag=pref + "sg", name=pref + "sg")
            nc.scalar.activation(sg[:], logit[:], AF.Sigmoid)
            time = slab.tile([128, T], F32, tag=pref + "tm", name=pref + "tm")
            a, c0 = (-esc_, -1.1) if neg else (esc_, 1.1)
            nc.vector.tensor_scalar(time[:], sg[:], a, c0, ALU.mult, ALU.add)
            lsq = slab.tile([128, T], F32, tag=pref + "lq", name=pref + "lq")
            nc.vector.tensor_tensor(lsq[:], logit[:], logit[:], ALU.mult)
            sq = slab.tile([128, T], F32, tag=pref + "sq", name=pref + "sq")
            nc.vector.tensor_tensor(sq[:], tot[:], lsq[:], ALU.subtract)
            sqc = slab.tile([128, T], F32, tag=pref + "sc", name=pref + "sc")
            nc.vector.tensor_scalar_max(sqc[:], sq[:], 1e-8)
            t2 = slab.tile([128, T], F32, tag=pref + "t2", name=pref + "t2")
            nc.vector.tensor_tensor(t2[:], time[:], time[:], ALU.mult)
            rec = slab.tile([128, T], F32, tag=pref + "rc", name=pref + "rc")
            nc.vector.reciprocal(rec[:], sqc[:])
            ratio = slab.tile([128, T], F32, tag=pref + "ra", name=pref + "ra")
            # ratio = (time^2 - 1) / sq_spatial
            nc.vector.scalar_tensor_tensor(ratio[:], t2[:], -1.0, rec[:],
                                           ALU.add, ALU.mult)
            rsq = slab.tile([128, T], F32, tag=pref + "rq", name=pref + "rq")
            fast_rsqrt(rsq[:], ratio[:], slab, T, pref + "fq")
            sqs = slab.tile([128, T], F32, tag=pref + "ss", name=pref + "ss")
            nc.vector.tensor_tensor(sqs[:], ratio[:], rsq[:], ALU.mult)
            return time, sqs

        def finish_array(T, dest3, time, sqs, halves=2):
            """Scale spatial cols in place (broadcast multiply), then
            overwrite col 0 with time. Split into halves so downstream
            transposes can start on the first half early."""
            step = T // halves
            for hh in range(halves):
                s0 = hh * step
                nc.vector.tensor_tensor(
                    dest3[:, s0:s0 + step, 0:64], dest3[:, s0:s0 + step, 0:64],
                    sqs[:, s0:s0 + step].to_broadcast((128, step, 64)),
                    ALU.mult)
                nc.vector.tensor_copy(dest3[:, s0:s0 + step, 0],
                                      time[:, s0:s0 + step])

        # ---- phase A0: hq (local rows; independent of everything) ----
        hqpad = oneshot.tile([128, TL * 128], BF16, tag="hq")
        hqpad3 = hqpad.rearrange("p (t c) -> p t c", c=128)
        tm_hq, ss_hq = linear_array(
            TL, lambda t: xqT_s[:, t * 128:(t + 1) * 128], wT_s[:],
            esc, False, hqpad3, "hq")
        finish_array(TL, hqpad3, tm_hq, ss_hq, halves=1)
        nc.vector.memset(hqpad3[:, :, 64], 1.0)
        hqT_flat = oneshot.tile([128, TL * 128], BF16, tag="hqT")
        nc.scalar.dma_start(hqT_flat.rearrange("p (t n) -> p t n", n=128),
                            hqpad[:], transpose=True)

        # ---- phase A: h (all rows) -----------------------------------
        tm_h, ss_h = linear_array(
            TJ, lambda t: xT_s[:, t * 128:(t + 1) * 128], wT_s[:],
            esc, False, hpad3, "h")

        # ---- phase Bq: qm (local rows) -------------------------------
        qm_pad = oneshot.tile([128, TL * 128], BF16, tag="qmpad")
        qm_pad3 = qm_pad.rearrange("p (t c) -> p t c", c=128)
        tm_qm, ss_qm = linear_array(
            TL, lambda t: hqT_flat[0:65, t * 128:(t + 1) * 128], wqT_s[:],
            esc_q, True, qm_pad3, "qm")
        finish_array(TL, qm_pad3, tm_qm, ss_qm, halves=1)
        nc.vector.tensor_copy(qm_pad3[:, :, 64:128], qm_pad3[:, :, 0:64])
        nc.scalar.dma_start(qmT_full.rearrange("p (t n) -> p t n", n=128),
                            qm_pad[:], transpose=True)

        # ---- finish h + transpose (split halves across 2 DMA queues) -
        finish_array(TJ, hpad3, tm_h, ss_h, halves=2)
        nc.vector.memset(hpad3[:, :, 64], 1.0)
        hT_flat = flat.tile([128, TJ * 128], BF16, tag="flat")
        hT3 = hT_flat.rearrange("p (t n) -> p t n", n=128)
        HH = TJ // 2
        nc.sync.dma_start(hT3[:, 0:HH, :], hpad[:, 0:HH * 128],
                          transpose=True)
        nc.scalar.dma_start(hT3[:, HH:TJ, :], hpad[:, HH * 128:TJ * 128],
                            transpose=True)

        # ---- phase B: k (all rows) -----------------------------------
        kdense = flat.tile([128, TJ * 64], BF16, tag="flat")
        kdense3 = kdense.rearrange("p (t d) -> p t d", d=64)
        tm_k, ss_k = linear_array(
            TJ, lambda t: hT_flat[0:65, t * 128:(t + 1) * 128], wkT_s[:],
            esc_k, False, kdense3, "k")
        finish_array(TJ, kdense3, tm_k, ss_k, halves=2)
        kT3 = kT_stk.rearrange("p (t n) -> p t n", n=128)
        KH = TJ // 4
        nc.sync.dma_start(kT3[:, 0:KH, :], kdense[:, 0:KH * 128],
                          transpose=True)
        nc.scalar.dma_start(kT3[:, KH:2 * KH, :],
                            kdense[:, KH * 128:2 * KH * 128],
                            transpose=True)

    # =========== phase C: attention + support =========================
    # adjt layout: [NCH*NHEX*128, 6*512] bf16 -- row ((c*NHEX+g)*128+p),
    # col (t*512+q) = adjT[( g*6+t)*128+p, c*512+q]: each partition's 6
    # j-subtiles are CONTIGUOUS 6KB in DRAM.
    adjt2 = io["adjt"]

    with tc.tile_pool(name="psA", bufs=2, space="PSUM") as psA, \
         tc.tile_pool(name="psS", bufs=1, space="PSUM") as psS:

        for c in range(NCH):
            supT = psS.tile([128, 512], F32, tag="supT", name="supT")
            qch = qmT_full[:, c * IC:(c + 1) * IC]
            prev_lo = prev_hi = None
            tris = {}       # triple idx -> psum tile
            sigm = {}       # j -> masked sig slice (MM2 rhs)
            pend = []       # even-j MM2 pairs awaiting sigm

            def att_dest(j):
                t = j // 3
                if t not in tris:
                    tris[t] = psA.tile([128, 1536], F32, tag="attT",
                                       name="attT")
                sl = j % 3
                return tris[t][:, sl * 512:(sl + 1) * 512]

            def emit_mm2(jl):
                nonlocal prev_lo, prev_hi
                start = jl == 0
                stop = jl == TJ - 2
                sA = nc.tensor.matmul(supT[0:64, :], hpad3[:, jl, 0:64],
                                      sigm[jl], start=start, stop=stop,
                                      tile_position=(0, 0))
                if prev_lo is not None:
                    add_dep_helper(sA.ins, prev_lo.ins, sync=False,
                                   reason="supT lo accum order")
                prev_lo = sA
                sB = nc.tensor.matmul(supT[64:128, :], hpad3[:, jl + 1, 0:64],
                                      sigm[jl + 1], start=start, stop=stop,
                                      tile_position=(0, 64))
                if prev_hi is not None:
                    add_dep_helper(sB.ins, prev_hi.ins, sync=False,
                                   reason="supT hi accum order")
                prev_hi = sB

            for g in range(NHEX):
                hx = hex_pool.tile([128, 6 * 512], BF16, tag="hex", name="hx")
                hx3 = hx.rearrange("p (t q) -> p t q", q=512)
                r0 = (c * NHEX + g) * 128
                nc.sync.dma_start(hx[:], adjt2[r0:r0 + 128, :])
                for pr in range(3):          # 3 j-pairs per hex
                    j0 = g * 6 + pr * 2
                    d0 = att_dest(j0)
                    d1 = att_dest(j0 + 1)
                    tp = j0 // 2
                    nc.tensor.matmul(d0, kT_stk[0:64, tp * 128:(tp + 1) * 128],
                                     qch[0:64, :], start=True, stop=True,
                                     tile_position=(0, 0))
                    nc.tensor.matmul(d1, kT_stk[64:128,
                                                tp * 128:(tp + 1) * 128],
                                     qch[64:128, :], start=True, stop=True,
                                     tile_position=(64, 0))
                    for jj in (j0, j0 + 1):
                        if jj % 3 == 2:      # triple complete -> sigmoid+mask
                            t = jj // 3
                            sig_t = sig_pool.tile([128, 1536], BF16,
                                                  tag="sig", name="sig_t")
                            nc.scalar.activation(sig_t[:], tris[t][:],
                                                 AF.Sigmoid,
                                                 bias=sig_bias_ap[:],
                                                 scale=sig_scale)
                            sm = sigm_pool.tile([128, 1536], BF16, tag="sm",
                                                name="sm")
                            # mask: multiply by adjT (hex cols are triple-
                            # aligned: triple t covers hex cols (3t%6)..+2)
                            hc = (3 * t) % 6
                            nc.vector.tensor_tensor(
                                sm[:], sig_t[:],
                                hx[:, hc * 512:(hc + 3) * 512], ALU.mult)
                            for k in range(3):
                                sigm[3 * t + k] = sm[:, k * 512:(k + 1) * 512]
                    pend.append(j0)
                    while pend and (pend[0] + 1) in sigm:
                        emit_mm2(pend.pop(0))
            while pend:
                emit_mm2(pend.pop(0))

            # ---- normalize + write out this chunk ------------------------
            lo_s = small.tile([64, 512], F32, tag="los", name="lo_s")
            nc.vector.tensor_copy(lo_s[:], supT[0:64, :])
            sup_s = small.tile([64, 512], F32, tag="sups", name="sup_s")
            nc.vector.tensor_tensor(sup_s[:], supT[64:128, :], lo_s[:],
                                    ALU.add)
            sq_all = out_pool.tile([128, 4 * 64], F32, tag="sqall",
                                   name="sq_all")
            sq_all3 = sq_all.rearrange("p (s d) -> p s d", d=64)
            o_raw = out_pool.tile([128, 4 * 64], F32, tag="oraw",
                                  name="o_raw")
            o_raw3 = o_raw.rearrange("p (s d) -> p s d", d=64)
            for s in range(4):
                supn = psS.tile([128, 512], F32, tag="tp", name="supn")
                supn = supn[:, 0:64]
                nc.tensor.transpose(supn, sup_s[:, s * 128:(s + 1) * 128],
                                    ident[:])
                nc.scalar.activation(sq_all3[:, s, :], supn, AF.Square)
                nc.vector.tensor_copy(o_raw3[:, s, :], supn)
            tot4 = small.tile([128, 4], F32, tag="ftot", name="tot4")
            nc.vector.tensor_reduce(tot4[:], sq_all3,
                                    axis=mybir.AxisListType.X, op=ALU.add)
            inner4 = small.tile([128, 4], F32, tag="finn", name="inner4")
            # inner = tot - 2*s0^2  (= -s0^2 + sum_{d>=1} s_d^2)
            nc.vector.scalar_tensor_tensor(inner4[:], sq_all3[:, :, 0], -2.0,
                                           tot4[:], ALU.mult, ALU.add)
            negv = small.tile([128, 4], F32, tag="fneg", name="negv")
            nc.vector.tensor_scalar_mul(negv[:], inner4[:], -1.0)
            absv = small.tile([128, 4], F32, tag="fabs", name="absv")
            nc.vector.tensor_tensor(absv[:], inner4[:], negv[:], ALU.max)
            clip4 = small.tile([128, 4], F32, tag="fclip", name="clip4")
            nc.vector.tensor_scalar_max(clip4[:], absv[:], 1e-8)
            rs4 = small.tile([128, 4], F32, tag="frs", name="rs4")
            fast_rsqrt(rs4[:], clip4[:], small, 4, "ff")
            o_t = out_pool.tile([128, 4 * 64], F32, tag="otile", name="o_t")
            o_t3 = o_t.rearrange("p (s d) -> p s d", d=64)
            nc.vector.tensor_tensor(o_t3[:], o_raw3[:],
                                    rs4[:].to_broadcast((128, 4, 64)),
                                    ALU.mult)
            nc.sync.dma_start(
                io["out"][c * IC:(c + 1) * IC, :].rearrange(
                    "(s p) d -> p s d", p=128), o_t3[:])

    ctx.close()


def build(nn, rr, esc, esc_q, esc_k, sig_scale, sig_bias, num_devices=N_CORES):
    nc = bacc.Bacc("TRN2", target_bir_lowering=False, debug=False,
                   num_devices=num_devices)
    nch = 3
    nhex = nn // 128 // 6
    io = {
        "adjt": nc.dram_tensor("adjt", [nch * nhex * 128, 6 * 512], BF16,
                               kind="ExternalInput").ap(),
        "xT": nc.dram_tensor("xT", [65, nn], BF16, kind="ExternalInput").ap(),
        "xqT": nc.dram_tensor("xqT", [65, rr], BF16,
                              kind="ExternalInput").ap(),
        "wT": nc.dram_tensor("wT", [65, 64], BF16, kind="ExternalInput").ap(),
        "wqT": nc.dram_tensor("wqT", [65, 64], BF16,
                              kind="ExternalInput").ap(),
        "wkT": nc.dram_tensor("wkT", [65, 64], BF16,
                              kind="ExternalInput").ap(),
        "out": nc.dram_tensor("out", [rr, 64], F32, kind="ExternalOutput").ap(),
    }
    with tile.TileContext(nc) as tc:
        emit(tc, io, nn, rr, esc, esc_q, esc_k, sig_scale, sig_bias)
    nc.compile()
    return nc


def make_in_maps(inputs, nn, rr, n_cores):
    bf = ml_dtypes.bfloat16
    x = np.asarray(inputs["x"], np.float32)
    adj = np.ascontiguousarray(np.asarray(inputs["adj"], np.float32))
    W = np.asarray(inputs["W"], np.float32)
    b = np.asarray(inputs["b"], np.float32)
    Wq = np.asarray(inputs["Wq"], np.float32)
    bq = np.asarray(inputs["bq"], np.float32)
    Wk = np.asarray(inputs["Wk"], np.float32)
    bk = np.asarray(inputs["bk"], np.float32)

    xT_ext = np.concatenate([x.T, np.ones((1, nn), np.float32)], 0).astype(bf)
    wT_ext = np.concatenate([W.T, b[None, :]], 0).astype(bf)
    wqT_ext = np.concatenate([Wq.T, bq[None, :]], 0).astype(bf)
    wkT_ext = np.concatenate([Wk.T, bk[None, :]], 0).astype(bf)

    in_maps = []
    for c in range(n_cores):
        r0 = c * rr
        slab = adj[r0:r0 + rr]                       # [1536, 12288]
        # adjt[(ch*16+g)*128+p, t*512+q] = slab[ch*512+q, (g*6+t)*128+p]
        # (per-partition 6KB contiguous lines for the hex DMAs)
        a6 = slab.reshape(3, 512, 16, 6, 128).transpose(0, 2, 4, 3, 1)
        adjt = np.ascontiguousarray(a6.reshape(3 * 16 * 128, 6 * 512)).astype(bf)
        in_maps.append({
            "adjt": adjt,
            "xT": np.ascontiguousarray(xT_ext),
            "xqT": np.ascontiguousarray(xT_ext[:, r0:r0 + rr]),
            "wT": wT_ext,
            "wqT": wqT_ext,
            "wkT": wkT_ext,
        })
    return in_maps


def consts_from_inputs(inputs):
    scale = float(np.asarray(inputs["scale"], np.float32))
    scale_q = float(np.asarray(inputs["scale_q"], np.float32))
    scale_k = float(np.asarray(inputs["scale_k"], np.float32))
    att_bias = float(np.asarray(inputs["att_bias"], np.float32))
    att_scale = float(np.asarray(inputs["att_scale"], np.float32))
    esc = math.exp(scale)
    esc_q = math.exp(scale_q)
    esc_k = math.exp(scale_k)
    sig_scale = 2.0 / att_scale
    sig_bias = 2.0 / att_scale + att_bias
    return esc, esc_q, esc_k, sig_scale, sig_bias


def kernel(**inputs):
    nn, rr = N_FULL, R_FULL
    consts = consts_from_inputs(inputs)
    nc = build(nn, rr, *consts)
    in_maps = make_in_maps(inputs, nn, rr, N_CORES)
    res = bass_utils.run_bass_kernel_spmd(nc, in_maps,
                                          core_ids=list(range(N_CORES)))
    return np.concatenate([res.results[c]["out"] for c in range(N_CORES)],
                          axis=0)


# revision 16
# speedup vs baseline: 1.0671x; 1.0671x over previous
"""Trainium2 Bass kernel for nn_LorentzGraphConvolution (v5).

Row-sharded across 8 NeuronCores: core c owns rows [c*1536, (c+1)*1536) of
the attention matrix / output. Every core redundantly computes the tiny
linear phase (h, k for all N; q for its local rows) from broadcast inputs,
so no collectives are needed.

Phase C engine budget per core (the ACT sigmoid is the wall):
  - ACT: one Sigmoid ACTIVATE per j-PAIR [128,1024] from PSUM (~1.09us
    per 2 tiles -- FD=1024 is the measured sweet spot; wider tiles pay a
    superlinear overhead). The whole kernel uses ONE act-table set
    (sigmoid_and_others: sigmoid+square+copy) -> no table reloads.
  - PE: row-packed MM1 pairs (K=64+64 concurrent), additive-mask matmuls
    (attT += BIG*adjT, fp8), col-tiled MM2 pairs (M=64 outputs to PSUM
    partitions 0:64 / 64:128 concurrent).
  - DMA: adjT pre-transposed AND pre-cast to fp8 on the HOST (free),
    streamed in 8-tile octet groups with 4KB contiguous per-partition
    lines on the otherwise-idle GPSIMD (SWDGE) queue, so the Sync queue
    (input loads + transposes) never blocks adjacency prefetch.
  - PSUM: the linear phase's pool is scoped and released before phase C:
    attention pairs [128,1024] x3 bufs (6 banks) + supT (1) + transpose
    scratch (1) = 8 banks.
"""

import math
import os
import sys
from contextlib import ExitStack

for _p in ("/opt/trn_rl_repo", "/root/.axon_site/_ro/trn_rl_repo", "/root/.axon_site"):
    if os.path.isdir(_p) and _p not in sys.path:
        sys.path.insert(0, _p)

import ml_dtypes
import numpy as np

import concourse.bass as bass
import concourse.tile as tile
from concourse import bacc, bass_utils, masks, mybir
from concourse.tile import add_dep_helper

DT = mybir.dt
F32 = DT.float32
BF16 = DT.bfloat16
F8 = DT.float8e4
AF = mybir.ActivationFunctionType
ALU = mybir.AluOpType

N_FULL = 12288
D = 64
N_CORES = 8
R_FULL = N_FULL // N_CORES  # 1536 rows per core


def pick_big(sig_scale):
    """Smallest fp8_e4m3-exact value >= 45/sig_scale (pushes masked logits
    below sigmoid(-24) while staying exactly representable)."""
    want = 45.0 / sig_scale
    v = float(np.float32(ml_dtypes.float8_e4m3(want)))
    while v < want:
        want *= 1.0625
        v = float(np.float32(ml_dtypes.float8_e4m3(want)))
    return v


def emit(tc, io, nn, rr, esc, esc_q, esc_k, sig_scale, sig_bias, big):
    nc = tc.nc
    TJ = nn // 128          # 96 global j tiles
    TL = rr // 128          # 12 local i tiles
    NCH = 3                 # i-chunks per core
    IC = rr // NCH          # 512 rows per chunk
    NOCT = TJ // 8          # 12 octets of j tiles per chunk
    assert IC == 512 and TJ % 8 == 0

    ctx = ExitStack()

    const = ctx.enter_context(tc.tile_pool(name="const", bufs=1))
    persist = ctx.enter_context(tc.tile_pool(name="persist", bufs=1))
    slab = ctx.enter_context(tc.tile_pool(name="slab", bufs=1))
    flat = ctx.enter_context(tc.tile_pool(name="flat", bufs=2))
    oneshot = ctx.enter_context(tc.tile_pool(name="oneshot", bufs=1))
    wide = ctx.enter_context(tc.tile_pool(name="wide", bufs=2))
    small = ctx.enter_context(tc.tile_pool(name="small", bufs=4))
    oct_pool = ctx.enter_context(tc.tile_pool(name="octs", bufs=6))
    sig_pool = ctx.enter_context(tc.tile_pool(name="sig", bufs=4))
    out_pool = ctx.enter_context(tc.tile_pool(name="outp", bufs=4))

    # ---- constants / small inputs -------------------------------------
    xT_s = flat.tile([65, nn], BF16, tag="flat")
    NXS = 4
    for xs in range(NXS):
        w0 = xs * (nn // NXS)
        nc.sync.dma_start(xT_s[:, w0:w0 + nn // NXS],
                          io["xT"][:, w0:w0 + nn // NXS])
    xqT_s = const.tile([65, rr], BF16)
    nc.sync.dma_start(xqT_s[:], io["xqT"][:])
    wT_s = const.tile([65, 64], BF16)
    nc.sync.dma_start(wT_s[:], io["wT"][:])
    wqT_s = const.tile([65, 64], BF16)
    nc.sync.dma_start(wqT_s[:], io["wqT"][:])
    wkT_s = const.tile([65, 64], BF16)
    nc.sync.dma_start(wkT_s[:], io["wkT"][:])
    bigI = const.tile([128, 128], F8)
    nc.sync.dma_start(bigI[:], io["bigi"][:])
    ident = const.tile([64, 64], F32)
    masks.make_identity(nc, ident[:])
    sig_bias_ap = const.tile([128, 1], F32)
    nc.vector.memset(sig_bias_ap[:], sig_bias - big * sig_scale)
    I32 = DT.int32
    magic = const.tile([128, 1], I32)
    nc.vector.memset(magic[:], 0x5F3759DF)

    def fast_rsqrt(dst, x, tmp_pool, nb, tag):
        """dst = 1/sqrt(x) via bit-trick + 2 Newton iterations (DVE only;
        keeps Sqrt off the ACT engine so phase C never swaps act tables)."""
        xi = x.bitcast(I32)
        sh = tmp_pool.tile([128, nb], I32, tag=tag + "sh", name="sh", bufs=2)
        nc.vector.tensor_scalar(sh[:], xi, 1, None, ALU.arith_shift_right)
        y = dst
        nc.vector.tensor_tensor(y.bitcast(I32), magic[:].to_broadcast((128, nb)),
                                sh[:], ALU.subtract)
        for _ in range(2):
            ysq = tmp_pool.tile([128, nb], F32, tag=tag + "ysq", name="ysq",
                                bufs=2)
            nc.vector.tensor_tensor(ysq[:], y, y, ALU.mult)
            t = tmp_pool.tile([128, nb], F32, tag=tag + "t", name="t", bufs=2)
            nc.vector.tensor_tensor(t[:], ysq[:], x, ALU.mult)
            w = tmp_pool.tile([128, nb], F32, tag=tag + "w", name="w", bufs=2)
            nc.vector.tensor_scalar(w[:], t[:], -0.5, 1.5, ALU.mult, ALU.add)
            yn = tmp_pool.tile([128, nb], F32, tag=tag + "yn", name="yn",
                               bufs=2)
            nc.vector.tensor_tensor(yn[:], y, w[:], ALU.mult)
            y = yn[:]
        nc.vector.tensor_copy(dst, y)

    # persistent per-core tensors. "pad" slabs put tile t's 64 features in
    # cols [t*128, t*128+64) so a 128x128 block DMA-transpose lands the
    # features at partitions 0:64; col 64 holds the bias-ones row.
    hpad = persist.tile([128, TJ * 128], BF16)
    hpad3 = hpad.rearrange("p (t c) -> p t c", c=128)
    # k^T stacked pairs: block t' rows 0:64 = kT[2t'], rows 64:128 = kT[2t'+1]
    kT_stk = persist.tile([128, (TJ // 2) * 128], BF16)
    # qm^T duplicated in both partition halves for the row-packed MM1 pairs
    qmT_full = persist.tile([128, TL * 128], BF16)

    # =========== linear phase (scoped PSUM pool) ======================
    with tc.tile_pool(name="psL", bufs=2, space="PSUM") as psL:

        def linear_array(T, lhsT_fn, rhs_w, esc_, neg, dest3, pref):
            """dest3: [128, T, c] bf16 view; after finish_array:
            col 0 = time, cols 1:64 = scaled spatial."""
            sq = slab.tile([128, T], F32, tag=pref + "sq", name=pref + "sq")
            logit = slab.tile([128, T], F32, tag=pref + "lg", name=pref + "lg")
            NB = 16
            for b0 in range(0, T, NB):
                nb = min(NB, T - b0)
                ps = psL.tile([128, 1024], F32, tag="linps", name="linps")
                ps = ps[:, : nb * 64]
                ps3 = ps.rearrange("p (t d) -> p t d", d=64)
                for u in range(nb):
                    nc.tensor.matmul(ps[:, u * 64:(u + 1) * 64],
                                     lhsT_fn(b0 + u), rhs_w, start=True,
                                     stop=True)
                sqf = wide.tile([128, 1024], F32, tag="sqw", name="sqw")
                sqf = sqf[:, : nb * 64]
                nc.scalar.activation(sqf, ps, AF.Square)
                sqf3 = sqf.rearrange("p (t d) -> p t d", d=64)
                # spatial sum of squares directly (features 1..63)
                nc.vector.tensor_reduce(sq[:, b0:b0 + nb], sqf3[:, :, 1:64],
                                        axis=mybir.AxisListType.X, op=ALU.add)
                nc.vector.tensor_copy(logit[:, b0:b0 + nb], ps3[:, :, 0])
                # psum->slab copy on ACT (Copy shares the sigmoid table set)
                nc.scalar.activation(dest3[:, b0:b0 + nb, 0:64], ps3, AF.Copy)
            # slab-level Lorentz normalization
            sg = slab.tile([128, T], F32, tag=pref + "sg", name=pref + "sg")
            nc.scalar.activation(sg[:], logit[:], AF.Sigmoid)
            time = slab.tile([128, T], F32, tag=pref + "tm", name=pref + "tm")
            a, c0 = (-esc_, -1.1) if neg else (esc_, 1.1)
            nc.vector.tensor_scalar(time[:], sg[:], a, c0, ALU.mult, ALU.add)
            sqc = slab.tile([128, T], F32, tag=pref + "sc", name=pref + "sc")
            nc.vector.tensor_scalar_max(sqc[:], sq[:], 1e-8)
            t2 = slab.tile([128, T], F32, tag=pref + "t2", name=pref + "t2")
            nc.vector.tensor_tensor(t2[:], time[:], time[:], ALU.mult)
            rec = slab.tile([128, T], F32, tag=pref + "rc", name=pref + "rc")
            nc.vector.reciprocal(rec[:], sqc[:])
            ratio = slab.tile([128, T], F32, tag=pref + "ra", name=pref + "ra")
            # ratio = (time^2 - 1) / sq_spatial
            nc.vector.scalar_tensor_tensor(ratio[:], t2[:], -1.0, rec[:],
                                           ALU.add, ALU.mult)
            rsq = slab.tile([128, T], F32, tag=pref + "rq", name=pref + "rq")
            fast_rsqrt(rsq[:], ratio[:], slab, T, pref + "fq")
            sqs = slab.tile([128, T], F32, tag=pref + "ss", name=pref + "ss")
            nc.vector.tensor_tensor(sqs[:], ratio[:], rsq[:], ALU.mult)
            return time, sqs

        def finish_half(dest3, time, sqs, s0, step, ones_col):
            nc.vector.tensor_tensor(
                dest3[:, s0:s0 + step, 0:64], dest3[:, s0:s0 + step, 0:64],
                sqs[:, s0:s0 + step].to_broadcast((128, step, 64)), ALU.mult)
            nc.vector.tensor_copy(dest3[:, s0:s0 + step, 0],
                                  time[:, s0:s0 + step])
            if ones_col:
                nc.vector.memset(dest3[:, s0:s0 + step, 64], 1.0)

        # ---- phase A0: hq (local rows; independent of h) -------------
        hqpad = oneshot.tile([128, TL * 128], BF16, tag="hq")
        hqpad3 = hqpad.rearrange("p (t c) -> p t c", c=128)
        tm_hq, ss_hq = linear_array(
            TL, lambda t: xqT_s[:, t * 128:(t + 1) * 128], wT_s[:],
            esc, False, hqpad3, "hq")
        finish_half(hqpad3, tm_hq, ss_hq, 0, TL, True)
        hqT_flat = oneshot.tile([128, TL * 128], BF16, tag="hqT")
        nc.sync.dma_start(hqT_flat.rearrange("p (t n) -> p t n", n=128),
                          hqpad[:], transpose=True)

        # ---- phase A: h (all rows) -----------------------------------
        tm_h, ss_h = linear_array(
            TJ, lambda t: xT_s[:, t * 128:(t + 1) * 128], wT_s[:],
            esc, False, hpad3, "h")

        # ---- phase Bq: qm (local rows) -------------------------------
        qm_pad = oneshot.tile([128, TL * 128], BF16, tag="qmpad")
        qm_pad3 = qm_pad.rearrange("p (t c) -> p t c", c=128)
        tm_qm, ss_qm = linear_array(
            TL, lambda t: hqT_flat[0:65, t * 128:(t + 1) * 128], wqT_s[:],
            esc_q, True, qm_pad3, "qm")
        finish_half(qm_pad3, tm_qm, ss_qm, 0, TL, False)
        nc.vector.tensor_copy(qm_pad3[:, :, 64:128], qm_pad3[:, :, 0:64])
        nc.sync.dma_start(qmT_full.rearrange("p (t n) -> p t n", n=128),
                          qm_pad[:], transpose=True)

        # ---- finish h + transpose (halves pipelined on one queue) ----
        hT_flat = flat.tile([128, TJ * 128], BF16, tag="flat")
        hT3 = hT_flat.rearrange("p (t n) -> p t n", n=128)
        HH = TJ // 2
        for hh in range(2):
            s0 = hh * HH
            finish_half(hpad3, tm_h, ss_h, s0, HH, True)
            nc.sync.dma_start(hT3[:, s0:s0 + HH, :],
                              hpad[:, s0 * 128:(s0 + HH) * 128],
                              transpose=True)

        # ---- phase B: k (all rows) -----------------------------------
        kdense = flat.tile([128, TJ * 64], BF16, tag="flat")
        kdense3 = kdense.rearrange("p (t d) -> p t d", d=64)
        tm_k, ss_k = linear_array(
            TJ, lambda t: hT_flat[0:65, t * 128:(t + 1) * 128], wkT_s[:],
            esc_k, False, kdense3, "k")
        kT3 = kT_stk.rearrange("p (t n) -> p t n", n=128)
        KH = TJ // 4
        for hh in range(2):
            s0 = hh * HH
            finish_half(kdense3, tm_k, ss_k, s0, HH, False)
            nc.sync.dma_start(kT3[:, hh * KH:(hh + 1) * KH, :],
                              kdense[:, s0 * 64:(s0 + HH) * 64],
                              transpose=True)

    # =========== phase C: attention + support =========================
    # adjt layout: [NCH*NOCT*128, 8*512] fp8 -- row ((c*NOCT+o)*128+p),
    # col (t*512+q) = adjT[(o*8+t)*128+p, c*512+q]: each partition's 8
    # j-subtiles are CONTIGUOUS 4KB in DRAM. Streamed on the GPSIMD
    # (SWDGE) queue so Sync-queue transpose waits never block prefetch.
    adjt2 = io["adjt"]

    with tc.tile_pool(name="psA", bufs=3, space="PSUM") as psA, \
         tc.tile_pool(name="psS", bufs=1, space="PSUM") as psS:

        for c in range(NCH):
            supT = psS.tile([128, 512], F32, tag="supT", name="supT")
            qch = qmT_full[:, c * IC:(c + 1) * IC]
            pending = None
            prev_lo = prev_hi = None

            def emit_mm2(pend, stop):
                nonlocal prev_lo, prev_hi
                sig_t, jl = pend
                start = jl == 0
                sA = nc.tensor.matmul(supT[0:64, :], hpad3[:, jl, 0:64],
                                      sig_t[:, 0:512], start=start, stop=stop,
                                      tile_position=(0, 0))
                if prev_lo is not None:
                    add_dep_helper(sA.ins, prev_lo.ins, sync=False,
                                   reason="supT lo accum order")
                prev_lo = sA
                sB = nc.tensor.matmul(supT[64:128, :], hpad3[:, jl + 1, 0:64],
                                      sig_t[:, 512:1024], start=start,
                                      stop=stop, tile_position=(0, 64))
                if prev_hi is not None:
                    add_dep_helper(sB.ins, prev_hi.ins, sync=False,
                                   reason="supT hi accum order")
                prev_hi = sB

            for o in range(NOCT):
                oct = oct_pool.tile([128, 8 * 512], F8, tag="oct", name="oct")
                oct3 = oct.rearrange("p (t q) -> p t q", q=512)
                r0 = (c * NOCT + o) * 128
                nc.gpsimd.dma_start(oct[:], adjt2[r0:r0 + 128, :])
                for pr in range(4):
                    jl = o * 8 + pr * 2
                    tp = jl // 2
                    attT = psA.tile([128, 1024], F32, tag="attT", name="attT")
                    mmA = nc.tensor.matmul(attT[:, 0:512],
                                           kT_stk[0:64,
                                                  tp * 128:(tp + 1) * 128],
                                           qch[0:64, :], start=True,
                                           stop=False, tile_position=(0, 0))
                    mmB = nc.tensor.matmul(attT[:, 512:1024],
                                           kT_stk[64:128,
                                                  tp * 128:(tp + 1) * 128],
                                           qch[64:128, :], start=True,
                                           stop=False, tile_position=(64, 0))
                    mA = nc.tensor.matmul(attT[:, 0:512], bigI[:],
                                          oct3[:, 2 * pr, :], start=False,
                                          stop=True)
                    add_dep_helper(mA.ins, mmA.ins, sync=False,
                                   reason="mask after ip A")
                    mB = nc.tensor.matmul(attT[:, 512:1024], bigI[:],
                                          oct3[:, 2 * pr + 1, :], start=False,
                                          stop=True)
                    add_dep_helper(mB.ins, mmB.ins, sync=False,
                                   reason="mask after ip B")
                    sig_t = sig_pool.tile([128, 1024], BF16, tag="sig",
                                          name="sig_t")
                    nc.scalar.activation(sig_t[:], attT[:], AF.Sigmoid,
                                         bias=sig_bias_ap[:], scale=sig_scale)
                    if pending is not None:
                        emit_mm2(pending, stop=False)
                    pending = (sig_t[:], jl)
            emit_mm2(pending, stop=True)

            # ---- normalize + write out this chunk --------------------
            lo_s = small.tile([64, 512], F32, tag="los", name="lo_s")
            nc.vector.tensor_copy(lo_s[:], supT[0:64, :])
            sup_s = small.tile([64, 512], F32, tag="sups", name="sup_s")
            nc.vector.tensor_tensor(sup_s[:], supT[64:128, :], lo_s[:],
                                    ALU.add)
            sq_all = out_pool.tile([128, 4 * 64], F32, tag="sqall",
                                   name="sq_all")
            sq_all3 = sq_all.rearrange("p (s d) -> p s d", d=64)
            o_raw = out_pool.tile([128, 4 * 64], F32, tag="oraw",
                                  name="o_raw")
            o_raw3 = o_raw.rearrange("p (s d) -> p s d", d=64)
            for s in range(4):
                supn = psS.tile([128, 512], F32, tag="tp", name="supn")
                supn = supn[:, 0:64]
                nc.tensor.transpose(supn, sup_s[:, s * 128:(s + 1) * 128],
                                    ident[:])
                nc.scalar.activation(sq_all3[:, s, :], supn, AF.Square)
                nc.vector.tensor_copy(o_raw3[:, s, :], supn)
            tot4 = small.tile([128, 4], F32, tag="ftot", name="tot4")
            nc.vector.tensor_reduce(tot4[:], sq_all3,
                                    axis=mybir.AxisListType.X, op=ALU.add)
            inner4 = small.tile([128, 4], F32, tag="finn", name="inner4")
            # inner = tot - 2*s0^2  (= -s0^2 + sum_{d>=1} s_d^2)
            nc.vector.scalar_tensor_tensor(inner4[:], sq_all3[:, :, 0], -2.0,
                                           tot4[:], ALU.mult, ALU.add)
            negv = small.tile([128, 4], F32, tag="fneg", name="negv")
            nc.vector.tensor_scalar_mul(negv[:], inner4[:], -1.0)
            absv = small.tile([128, 4], F32, tag="fabs", name="absv")
            nc.vector.tensor_tensor(absv[:], inner4[:], negv[:], ALU.max)
            clip4 = small.tile([128, 4], F32, tag="fclip", name="clip4")
            nc.vector.tensor_scalar_max(clip4[:], absv[:], 1e-8)
            rs4 = small.tile([128, 4], F32, tag="frs", name="rs4")
            fast_rsqrt(rs4[:], clip4[:], small, 4, "ff")
            o_t = out_pool.tile([128, 4 * 64], F32, tag="otile", name="o_t")
            o_t3 = o_t.rearrange("p (s d) -> p s d", d=64)
            nc.vector.tensor_tensor(o_t3[:], o_raw3[:],
                                    rs4[:].to_broadcast((128, 4, 64)),
                                    ALU.mult)
            nc.sync.dma_start(
                io["out"][c * IC:(c + 1) * IC, :].rearrange(
                    "(s p) d -> p s d", p=128), o_t3[:])

    ctx.close()


def build(nn, rr, esc, esc_q, esc_k, sig_scale, sig_bias, num_devices=N_CORES):
    big = pick_big(sig_scale)
    nc = bacc.Bacc("TRN2", target_bir_lowering=False, debug=False,
                   num_devices=num_devices)
    nch = 3
    noct = nn // 128 // 8
    io = {
        "adjt": nc.dram_tensor("adjt", [nch * noct * 128, 8 * 512], F8,
                               kind="ExternalInput").ap(),
        "xT": nc.dram_tensor("xT", [65, nn], BF16, kind="ExternalInput").ap(),
        "xqT": nc.dram_tensor("xqT", [65, rr], BF16,
                              kind="ExternalInput").ap(),
        "wT": nc.dram_tensor("wT", [65, 64], BF16, kind="ExternalInput").ap(),
        "wqT": nc.dram_tensor("wqT", [65, 64], BF16,
                              kind="ExternalInput").ap(),
        "wkT": nc.dram_tensor("wkT", [65, 64], BF16,
                              kind="ExternalInput").ap(),
        "bigi": nc.dram_tensor("bigi", [128, 128], F8,
                               kind="ExternalInput").ap(),
        "out": nc.dram_tensor("out", [rr, 64], F32, kind="ExternalOutput").ap(),
    }
    with tile.TileContext(nc) as tc:
        emit(tc, io, nn, rr, esc, esc_q, esc_k, sig_scale, sig_bias, big)
    nc.compile()
    return nc


def make_in_maps(inputs, nn, rr, n_cores):
    bf = ml_dtypes.bfloat16
    f8 = ml_dtypes.float8_e4m3
    x = np.asarray(inputs["x"], np.float32)
    adj = np.ascontiguousarray(np.asarray(inputs["adj"], np.float32))
    W = np.asarray(inputs["W"], np.float32)
    b = np.asarray(inputs["b"], np.float32)
    Wq = np.asarray(inputs["Wq"], np.float32)
    bq = np.asarray(inputs["bq"], np.float32)
    Wk = np.asarray(inputs["Wk"], np.float32)
    bk = np.asarray(inputs["bk"], np.float32)

    att_scale = float(np.asarray(inputs["att_scale"], np.float32))
    big = pick_big(2.0 / att_scale)

    xT_ext = np.concatenate([x.T, np.ones((1, nn), np.float32)], 0).astype(bf)
    wT_ext = np.concatenate([W.T, b[None, :]], 0).astype(bf)
    wqT_ext = np.concatenate([Wq.T, bq[None, :]], 0).astype(bf)
    wkT_ext = np.concatenate([Wk.T, bk[None, :]], 0).astype(bf)
    bigI = (np.eye(128, dtype=np.float32) * big).astype(f8)

    in_maps = []
    for c in range(n_cores):
        r0 = c * rr
        slab = adj[r0:r0 + rr]                       # [1536, 12288]
        # adjt[(ch*12+o)*128+p, t*512+q] = slab[ch*512+q, (o*8+t)*128+p]
        # (per-partition 4KB contiguous lines for the octet DMAs)
        a6 = slab.reshape(3, 512, 12, 8, 128).transpose(0, 2, 4, 3, 1)
        adjt = np.ascontiguousarray(a6.reshape(3 * 12 * 128, 8 * 512)).astype(f8)
        in_maps.append({
            "adjt": adjt,
            "xT": np.ascontiguousarray(xT_ext),
            "xqT": np.ascontiguousarray(xT_ext[:, r0:r0 + rr]),
            "wT": wT_ext,
            "wqT": wqT_ext,
            "wkT": wkT_ext,
            "bigi": bigI,
        })
    return in_maps


def consts_from_inputs(inputs):
    scale = float(np.asarray(inputs["scale"], np.float32))
    scale_q = float(np.asarray(inputs["scale_q"], np.float32))
    scale_k = float(np.asarray(inputs["scale_k"], np.float32))
    att_bias = float(np.asarray(inputs["att_bias"], np.float32))
    att_scale = float(np.asarray(inputs["att_scale"], np.float32))
    esc = math.exp(scale)
    esc_q = math.exp(scale_q)
    esc_k = math.exp(scale_k)
    sig_scale = 2.0 / att_scale
    sig_bias = 2.0 / att_scale + att_bias
    return esc, esc_q, esc_k, sig_scale, sig_bias


def kernel(**inputs):
    nn, rr = N_FULL, R_FULL
    consts = consts_from_inputs(inputs)
    nc = build(nn, rr, *consts)
    in_maps = make_in_maps(inputs, nn, rr, N_CORES)
    res = bass_utils.run_bass_kernel_spmd(nc, in_maps,
                                          core_ids=list(range(N_CORES)))
    return np.concatenate([res.results[c]["out"] for c in range(N_CORES)],
                          axis=0)


# revision 20
# speedup vs baseline: 1.2960x; 1.2145x over previous
"""Trainium2 Bass kernel for nn_LorentzGraphConvolution (v5).

Row-sharded across 8 NeuronCores: core c owns rows [c*1536, (c+1)*1536) of
the attention matrix / output. Every core redundantly computes the tiny
linear phase (h, k for all N; q for its local rows) from broadcast inputs,
so no collectives are needed.

Phase C engine budget per core (the ACT sigmoid is the wall):
  - ACT: one Sigmoid ACTIVATE per j-PAIR [128,1024] from PSUM (~1.09us
    per 2 tiles -- FD=1024 is the measured sweet spot; wider tiles pay a
    superlinear overhead). The whole kernel uses ONE act-table set
    (sigmoid_and_others: sigmoid+square+copy) -> no table reloads.
  - PE: row-packed MM1 pairs (K=64+64 concurrent), additive-mask matmuls
    (attT += BIG*adjT, fp8), col-tiled MM2 pairs (M=64 outputs to PSUM
    partitions 0:64 / 64:128 concurrent).
  - DMA: adjT pre-transposed AND pre-cast to fp8 on the HOST (free),
    streamed in 8-tile octet groups with 4KB contiguous per-partition
    lines on the otherwise-idle GPSIMD (SWDGE) queue, so the Sync queue
    (input loads + transposes) never blocks adjacency prefetch.
  - PSUM: the linear phase's pool is scoped and released before phase C:
    attention pairs [128,1024] x3 bufs (6 banks) + supT (1) + transpose
    scratch (1) = 8 banks.
"""

import math
import os
import sys
from contextlib import ExitStack

for _p in ("/opt/trn_rl_repo", "/root/.axon_site/_ro/trn_rl_repo", "/root/.axon_site"):
    if os.path.isdir(_p) and _p not in sys.path:
        sys.path.insert(0, _p)

import ml_dtypes
import numpy as np

import concourse.bass as bass
import concourse.tile as tile
from concourse import bacc, bass_utils, masks, mybir
from concourse.tile import add_dep_helper

DT = mybir.dt
F32 = DT.float32
BF16 = DT.bfloat16
F8 = DT.float8e4
AF = mybir.ActivationFunctionType
ALU = mybir.AluOpType

N_FULL = 12288
D = 64
N_CORES = 8
R_FULL = N_FULL // N_CORES  # 1536 rows per core


def pick_big(sig_scale):
    """Smallest fp8_e4m3-exact value >= 45/sig_scale (pushes masked logits
    below sigmoid(-24) while staying exactly representable)."""
    want = 45.0 / sig_scale
    v = float(np.float32(ml_dtypes.float8_e4m3(want)))
    while v < want:
        want *= 1.0625
        v = float(np.float32(ml_dtypes.float8_e4m3(want)))
    return v


def emit(tc, io, nn, rr, esc, esc_q, esc_k, sig_scale, sig_bias, big):
    nc = tc.nc
    TJ = nn // 128          # 96 global j tiles
    TL = rr // 128          # 12 local i tiles
    NCH = 3                 # i-chunks per core
    IC = rr // NCH          # 512 rows per chunk
    NOCT = TJ // 8          # 12 octets of j tiles per chunk
    assert IC == 512 and TJ % 8 == 0

    ctx = ExitStack()

    const = ctx.enter_context(tc.tile_pool(name="const", bufs=1))
    persist = ctx.enter_context(tc.tile_pool(name="persist", bufs=1))
    slab = ctx.enter_context(tc.tile_pool(name="slab", bufs=1))
    flat = ctx.enter_context(tc.tile_pool(name="flat", bufs=2))
    oneshot = ctx.enter_context(tc.tile_pool(name="oneshot", bufs=1))
    wide = ctx.enter_context(tc.tile_pool(name="wide", bufs=2))
    small = ctx.enter_context(tc.tile_pool(name="small", bufs=4))
    oct_pool = ctx.enter_context(tc.tile_pool(name="octs", bufs=6))
    sig_pool = ctx.enter_context(tc.tile_pool(name="sig", bufs=4))
    out_pool = ctx.enter_context(tc.tile_pool(name="outp", bufs=4))

    # ---- constants / small inputs -------------------------------------
    xT_s = flat.tile([65, nn], BF16, tag="flat")
    NXS = 4
    for xs in range(NXS):
        w0 = xs * (nn // NXS)
        nc.sync.dma_start(xT_s[:, w0:w0 + nn // NXS],
                          io["xT"][:, w0:w0 + nn // NXS])
    xqT_s = const.tile([65, rr], BF16)
    nc.sync.dma_start(xqT_s[:], io["xqT"][:])
    wT_s = const.tile([65, 64], BF16)
    nc.sync.dma_start(wT_s[:], io["wT"][:])
    wqT_s = const.tile([65, 64], BF16)
    nc.sync.dma_start(wqT_s[:], io["wqT"][:])
    wkT_s = const.tile([65, 64], BF16)
    nc.sync.dma_start(wkT_s[:], io["wkT"][:])
    bigI = const.tile([128, 128], F8)
    last_in_dma = nc.sync.dma_start(bigI[:], io["bigi"][:])
    ident = const.tile([64, 64], F32)
    masks.make_identity(nc, ident[:])
    sig_bias_big = const.tile([128, 1], F32)
    nc.vector.memset(sig_bias_big[:], sig_bias - big * sig_scale)
    sig_bias_ap = const.tile([128, 1], F32)
    nc.vector.memset(sig_bias_ap[:], sig_bias)
    I32 = DT.int32
    magic = const.tile([128, 1], I32)
    nc.vector.memset(magic[:], 0x5F3759DF)

    def fast_rsqrt(dst, x, tmp_pool, nb, tag):
        """dst = 1/sqrt(x) via bit-trick + 2 Newton iterations (DVE only;
        keeps Sqrt off the ACT engine so phase C never swaps act tables)."""
        xi = x.bitcast(I32)
        sh = tmp_pool.tile([128, nb], I32, tag=tag + "sh", name="sh", bufs=2)
        nc.vector.tensor_scalar(sh[:], xi, 1, None, ALU.arith_shift_right)
        y = dst
        nc.vector.tensor_tensor(y.bitcast(I32), magic[:].to_broadcast((128, nb)),
                                sh[:], ALU.subtract)
        for _ in range(2):
            ysq = tmp_pool.tile([128, nb], F32, tag=tag + "ysq", name="ysq",
                                bufs=2)
            nc.vector.tensor_tensor(ysq[:], y, y, ALU.mult)
            t = tmp_pool.tile([128, nb], F32, tag=tag + "t", name="t", bufs=2)
            nc.vector.tensor_tensor(t[:], ysq[:], x, ALU.mult)
            w = tmp_pool.tile([128, nb], F32, tag=tag + "w", name="w", bufs=2)
            nc.vector.tensor_scalar(w[:], t[:], -0.5, 1.5, ALU.mult, ALU.add)
            yn = tmp_pool.tile([128, nb], F32, tag=tag + "yn", name="yn",
                               bufs=2)
            nc.vector.tensor_tensor(yn[:], y, w[:], ALU.mult)
            y = yn[:]
        nc.vector.tensor_copy(dst, y)

    # persistent per-core tensors. "pad" slabs put tile t's 64 features in
    # cols [t*128, t*128+64) so a 128x128 block DMA-transpose lands the
    # features at partitions 0:64; col 64 holds the bias-ones row.
    hpad = persist.tile([128, TJ * 128], BF16)
    hpad3 = hpad.rearrange("p (t c) -> p t c", c=128)
    # k^T stacked pairs: block t' rows 0:64 = kT[2t'], rows 64:128 = kT[2t'+1]
    kT_stk = persist.tile([128, (TJ // 2) * 128], BF16)
    # qm^T duplicated in both partition halves for the row-packed MM1 pairs
    qmT_full = persist.tile([128, TL * 128], BF16)

    # =========== linear phase (scoped PSUM pool) ======================
    with tc.tile_pool(name="psL", bufs=2, space="PSUM") as psL:

        def linear_array(T, lhsT_fn, rhs_w, esc_, neg, dest3, pref):
            """dest3: [128, T, c] bf16 view; after finish_array:
            col 0 = time, cols 1:64 = scaled spatial."""
            sq = slab.tile([128, T], F32, tag=pref + "sq", name=pref + "sq")
            logit = slab.tile([128, T], F32, tag=pref + "lg", name=pref + "lg")
            NB = 16
            for b0 in range(0, T, NB):
                nb = min(NB, T - b0)
                ps = psL.tile([128, 1024], F32, tag="linps", name="linps")
                ps = ps[:, : nb * 64]
                ps3 = ps.rearrange("p (t d) -> p t d", d=64)
                for u in range(nb):
                    nc.tensor.matmul(ps[:, u * 64:(u + 1) * 64],
                                     lhsT_fn(b0 + u), rhs_w, start=True,
                                     stop=True)
                sqf = wide.tile([128, 1024], F32, tag="sqw", name="sqw")
                sqf = sqf[:, : nb * 64]
                nc.scalar.activation(sqf, ps, AF.Square)
                sqf3 = sqf.rearrange("p (t d) -> p t d", d=64)
                # spatial sum of squares directly (features 1..63)
                nc.vector.tensor_reduce(sq[:, b0:b0 + nb], sqf3[:, :, 1:64],
                                        axis=mybir.AxisListType.X, op=ALU.add)
                nc.vector.tensor_copy(logit[:, b0:b0 + nb], ps3[:, :, 0])
                # psum->slab copy on ACT (Copy shares the sigmoid table set)
                nc.scalar.activation(dest3[:, b0:b0 + nb, 0:64], ps3, AF.Copy)
            # slab-level Lorentz normalization
            sg = slab.tile([128, T], F32, tag=pref + "sg", name=pref + "sg")
            nc.scalar.activation(sg[:], logit[:], AF.Sigmoid)
            time = slab.tile([128, T], F32, tag=pref + "tm", name=pref + "tm")
            a, c0 = (-esc_, -1.1) if neg else (esc_, 1.1)
            nc.vector.tensor_scalar(time[:], sg[:], a, c0, ALU.mult, ALU.add)
            sqc = slab.tile([128, T], F32, tag=pref + "sc", name=pref + "sc")
            nc.vector.tensor_scalar_max(sqc[:], sq[:], 1e-8)
            t2 = slab.tile([128, T], F32, tag=pref + "t2", name=pref + "t2")
            nc.vector.tensor_tensor(t2[:], time[:], time[:], ALU.mult)
            rec = slab.tile([128, T], F32, tag=pref + "rc", name=pref + "rc")
            nc.vector.reciprocal(rec[:], sqc[:])
            ratio = slab.tile([128, T], F32, tag=pref + "ra", name=pref + "ra")
            # ratio = (time^2 - 1) / sq_spatial
            nc.vector.scalar_tensor_tensor(ratio[:], t2[:], -1.0, rec[:],
                                           ALU.add, ALU.mult)
            rsq = slab.tile([128, T], F32, tag=pref + "rq", name=pref + "rq")
            fast_rsqrt(rsq[:], ratio[:], slab, T, pref + "fq")
            sqs = slab.tile([128, T], F32, tag=pref + "ss", name=pref + "ss")
            nc.vector.tensor_tensor(sqs[:], ratio[:], rsq[:], ALU.mult)
            return time, sqs

        def finish_half(dest3, time, sqs, s0, step, ones_col, off=0):
            sl = slice(s0 + off, s0 + off + step)
            nc.vector.tensor_tensor(
                dest3[:, s0:s0 + step, 0:64], dest3[:, s0:s0 + step, 0:64],
                sqs[:, sl].to_broadcast((128, step, 64)), ALU.mult)
            nc.vector.tensor_copy(dest3[:, s0:s0 + step, 0], time[:, sl])
            if ones_col:
                nc.vector.memset(dest3[:, s0:s0 + step, 64], 1.0)

        # ---- phase A0: hq (local rows; independent of h) -------------
        hqpad = oneshot.tile([128, TL * 128], BF16, tag="hq")
        hqpad3 = hqpad.rearrange("p (t c) -> p t c", c=128)
        tm_hq, ss_hq = linear_array(
            TL, lambda t: xqT_s[:, t * 128:(t + 1) * 128], wT_s[:],
            esc, False, hqpad3, "hq")
        finish_half(hqpad3, tm_hq, ss_hq, 0, TL, True)
        hqT_flat = oneshot.tile([128, TL * 128], BF16, tag="hqT")
        nc.sync.dma_start(hqT_flat.rearrange("p (t n) -> p t n", n=128),
                          hqpad[:], transpose=True)

        # ---- phase A: h (all rows, two half-arrays) ------------------
        HH = TJ // 2
        hT_flat = flat.tile([128, TJ * 128], BF16, tag="flat")
        hT3 = hT_flat.rearrange("p (t n) -> p t n", n=128)
        h_halves = []
        for hh in range(2):
            s0 = hh * HH
            tm, ss = linear_array(
                HH, lambda t: xT_s[:, (s0 + t) * 128:(s0 + t + 1) * 128],
                wT_s[:], esc, False, hpad3[:, s0:s0 + HH, :], "h%d" % hh)
            h_halves.append((tm, ss))

        # ---- phase Bq: qm (local rows) -------------------------------
        qm_pad = oneshot.tile([128, TL * 128], BF16, tag="qmpad")
        qm_pad3 = qm_pad.rearrange("p (t c) -> p t c", c=128)
        tm_qm, ss_qm = linear_array(
            TL, lambda t: hqT_flat[0:65, t * 128:(t + 1) * 128], wqT_s[:],
            esc_q, True, qm_pad3, "qm")
        finish_half(qm_pad3, tm_qm, ss_qm, 0, TL, False)
        nc.vector.tensor_copy(qm_pad3[:, :, 64:128], qm_pad3[:, :, 0:64])
        nc.sync.dma_start(qmT_full.rearrange("p (t n) -> p t n", n=128),
                          qm_pad[:], transpose=True)

        # ---- finish h halves + transpose + k half-arrays -------------
        # (k's half starts as soon as the matching hT transpose lands)
        kdense = flat.tile([128, TJ * 64], BF16, tag="flat")
        kdense3 = kdense.rearrange("p (t d) -> p t d", d=64)
        kT3 = kT_stk.rearrange("p (t n) -> p t n", n=128)
        KH = TJ // 4
        for hh in range(2):
            s0 = hh * HH
            tm, ss = h_halves[hh]
            finish_half(hpad3, tm, ss, s0, HH, True, off=-s0)
            nc.sync.dma_start(hT3[:, s0:s0 + HH, :],
                              hpad[:, s0 * 128:(s0 + HH) * 128],
                              transpose=True)
            tmk, ssk = linear_array(
                HH, lambda t: hT_flat[0:65, (s0 + t) * 128:(s0 + t + 1) * 128],
                wkT_s[:], esc_k, False, kdense3[:, s0:s0 + HH, :], "k%d" % hh)
            finish_half(kdense3, tmk, ssk, s0, HH, False, off=-s0)
            nc.sync.dma_start(kT3[:, hh * KH:(hh + 1) * KH, :],
                              kdense[:, s0 * 64:(s0 + HH) * 64],
                              transpose=True)

    # =========== phase C: attention + support =========================
    # adjt layout: [NCH*NOCT*128, 8*512] fp8 -- row ((c*NOCT+o)*128+p),
    # col (t*512+q) = adjT[(o*8+t)*128+p, c*512+q]: each partition's 8
    # j-subtiles are CONTIGUOUS 4KB in DRAM. Streamed on the GPSIMD
    # (SWDGE) queue so Sync-queue transpose waits never block prefetch.
    adjt2 = io["adjt"]

    with tc.tile_pool(name="psA", bufs=3, space="PSUM") as psA, \
         tc.tile_pool(name="psS", bufs=1, space="PSUM") as psS:

        for c in range(NCH):
            supT = psS.tile([128, 512], F32, tag="supT", name="supT")
            qch = qmT_full[:, c * IC:(c + 1) * IC]
            pending = None
            prev_lo = prev_hi = None

            def emit_mm2(pend, stop):
                nonlocal prev_lo, prev_hi
                sig_t, jl = pend
                start = jl == 0
                sA = nc.tensor.matmul(supT[0:64, :], hpad3[:, jl, 0:64],
                                      sig_t[:, 0:512], start=start, stop=stop,
                                      tile_position=(0, 0))
                if prev_lo is not None:
                    add_dep_helper(sA.ins, prev_lo.ins, sync=False,
                                   reason="supT lo accum order")
                prev_lo = sA
                sB = nc.tensor.matmul(supT[64:128, :], hpad3[:, jl + 1, 0:64],
                                      sig_t[:, 512:1024], start=start,
                                      stop=stop, tile_position=(0, 64))
                if prev_hi is not None:
                    add_dep_helper(sB.ins, prev_hi.ins, sync=False,
                                   reason="supT hi accum order")
                prev_hi = sB

            for o in range(NOCT):
                oct = oct_pool.tile([128, 8 * 512], F8, tag="oct", name="oct")
                oct3 = oct.rearrange("p (t q) -> p t q", q=512)
                r0 = (c * NOCT + o) * 128
                odma = nc.gpsimd.dma_start(oct[:], adjt2[r0:r0 + 128, :])
                if c == 0 and o == 0:
                    # don't let adjacency prefetch race the startup loads
                    add_dep_helper(odma.ins, last_in_dma.ins, sync=True,
                                   reason="inputs before adj prefetch")
                for pr in range(4):
                    jl = o * 8 + pr * 2
                    tp = jl // 2
                    pe_mask = pr % 2 == 0
                    attT = psA.tile([128, 1024], F32, tag="attT", name="attT")
                    mmA = nc.tensor.matmul(attT[:, 0:512],
                                           kT_stk[0:64,
                                                  tp * 128:(tp + 1) * 128],
                                           qch[0:64, :], start=True,
                                           stop=not pe_mask,
                                           tile_position=(0, 0))
                    mmB = nc.tensor.matmul(attT[:, 512:1024],
                                           kT_stk[64:128,
                                                  tp * 128:(tp + 1) * 128],
                                           qch[64:128, :], start=True,
                                           stop=not pe_mask,
                                           tile_position=(64, 0))
                    if pe_mask:
                        # additive mask on PE: attT += BIG * adjT
                        mA = nc.tensor.matmul(attT[:, 0:512], bigI[:],
                                              oct3[:, 2 * pr, :], start=False,
                                              stop=True)
                        add_dep_helper(mA.ins, mmA.ins, sync=False,
                                       reason="mask after ip A")
                        mB = nc.tensor.matmul(attT[:, 512:1024], bigI[:],
                                              oct3[:, 2 * pr + 1, :],
                                              start=False, stop=True)
                        add_dep_helper(mB.ins, mmB.ins, sync=False,
                                       reason="mask after ip B")
                    sig_t = sig_pool.tile([128, 1024], BF16, tag="sig",
                                          name="sig_t")
                    nc.scalar.activation(
                        sig_t[:], attT[:], AF.Sigmoid,
                        bias=sig_bias_big[:] if pe_mask else sig_bias_ap[:],
                        scale=sig_scale)
                    if pe_mask:
                        rhs = sig_t[:]
                    else:
                        # multiplicative mask on DVE (idle in phase C)
                        sm = sig_pool.tile([128, 1024], BF16, tag="sm",
                                           name="sm")
                        nc.vector.tensor_tensor(
                            sm[:], sig_t[:],
                            oct[:, (2 * pr) * 512:(2 * pr + 2) * 512],
                            ALU.mult)
                        rhs = sm[:]
                    if pending is not None:
                        emit_mm2(pending, stop=False)
                    pending = (rhs, jl)
            emit_mm2(pending, stop=True)

            # ---- normalize + write out this chunk --------------------
            lo_s = small.tile([64, 512], F32, tag="los", name="lo_s")
            nc.vector.tensor_copy(lo_s[:], supT[0:64, :])
            sup_s = small.tile([64, 512], F32, tag="sups", name="sup_s")
            nc.vector.tensor_tensor(sup_s[:], supT[64:128, :], lo_s[:],
                                    ALU.add)
            sq_all = out_pool.tile([128, 4 * 64], F32, tag="sqall",
                                   name="sq_all")
            sq_all3 = sq_all.rearrange("p (s d) -> p s d", d=64)
            o_raw = out_pool.tile([128, 4 * 64], F32, tag="oraw",
                                  name="o_raw")
            o_raw3 = o_raw.rearrange("p (s d) -> p s d", d=64)
            for s in range(4):
                supn = psS.tile([128, 512], F32, tag="tp", name="supn")
                supn = supn[:, 0:64]
                nc.tensor.transpose(supn, sup_s[:, s * 128:(s + 1) * 128],
                                    ident[:])
                nc.scalar.activation(sq_all3[:, s, :], supn, AF.Square)
                nc.vector.tensor_copy(o_raw3[:, s, :], supn)
            tot4 = small.tile([128, 4], F32, tag="ftot", name="tot4")
            nc.vector.tensor_reduce(tot4[:], sq_all3,
                                    axis=mybir.AxisListType.X, op=ALU.add)
            inner4 = small.tile([128, 4], F32, tag="finn", name="inner4")
            # inner = tot - 2*s0^2  (= -s0^2 + sum_{d>=1} s_d^2)
            nc.vector.scalar_tensor_tensor(inner4[:], sq_all3[:, :, 0], -2.0,
                                           tot4[:], ALU.mult, ALU.add)
            negv = small.tile([128, 4], F32, tag="fneg", name="negv")
            nc.vector.tensor_scalar_mul(negv[:], inner4[:], -1.0)
            absv = small.tile([128, 4], F32, tag="fabs", name="absv")
            nc.vector.tensor_tensor(absv[:], inner4[:], negv[:], ALU.max)
            clip4 = small.tile([128, 4], F32, tag="fclip", name="clip4")
            nc.vector.tensor_scalar_max(clip4[:], absv[:], 1e-8)
            rs4 = small.tile([128, 4], F32, tag="frs", name="rs4")
            fast_rsqrt(rs4[:], clip4[:], small, 4, "ff")
            o_t = out_pool.tile([128, 4 * 64], F32, tag="otile", name="o_t")
            o_t3 = o_t.rearrange("p (s d) -> p s d", d=64)
            nc.vector.tensor_tensor(o_t3[:], o_raw3[:],
                                    rs4[:].to_broadcast((128, 4, 64)),
                                    ALU.mult)
            nc.sync.dma_start(
                io["out"][c * IC:(c + 1) * IC, :].rearrange(
                    "(s p) d -> p s d", p=128), o_t3[:])

    ctx.close()


def build(nn, rr, esc, esc_q, esc_k, sig_scale, sig_bias, num_devices=N_CORES):
    big = pick_big(sig_scale)
    nc = bacc.Bacc("TRN2", target_bir_lowering=False, debug=False,
                   num_devices=num_devices)
    nch = 3
    noct = nn // 128 // 8
    io = {
        "adjt": nc.dram_tensor("adjt", [nch * noct * 128, 8 * 512], F8,
                               kind="ExternalInput").ap(),
        "xT": nc.dram_tensor("xT", [65, nn], BF16, kind="ExternalInput").ap(),
        "xqT": nc.dram_tensor("xqT", [65, rr], BF16,
                              kind="ExternalInput").ap(),
        "wT": nc.dram_tensor("wT", [65, 64], BF16, kind="ExternalInput").ap(),
        "wqT": nc.dram_tensor("wqT", [65, 64], BF16,
                              kind="ExternalInput").ap(),
        "wkT": nc.dram_tensor("wkT", [65, 64], BF16,
                              kind="ExternalInput").ap(),
        "bigi": nc.dram_tensor("bigi", [128, 128], F8,
                               kind="ExternalInput").ap(),
        "out": nc.dram_tensor("out", [rr, 64], F32, kind="ExternalOutput").ap(),
    }
    with tile.TileContext(nc) as tc:
        emit(tc, io, nn, rr, esc, esc_q, esc_k, sig_scale, sig_bias, big)
    nc.compile()
    return nc


def make_in_maps(inputs, nn, rr, n_cores):
    bf = ml_dtypes.bfloat16
    f8 = ml_dtypes.float8_e4m3
    x = np.asarray(inputs["x"], np.float32)
    adj = np.ascontiguousarray(np.asarray(inputs["adj"], np.float32))
    W = np.asarray(inputs["W"], np.float32)
    b = np.asarray(inputs["b"], np.float32)
    Wq = np.asarray(inputs["Wq"], np.float32)
    bq = np.asarray(inputs["bq"], np.float32)
    Wk = np.asarray(inputs["Wk"], np.float32)
    bk = np.asarray(inputs["bk"], np.float32)

    att_scale = float(np.asarray(inputs["att_scale"], np.float32))
    big = pick_big(2.0 / att_scale)

    xT_ext = np.concatenate([x.T, np.ones((1, nn), np.float32)], 0).astype(bf)
    wT_ext = np.concatenate([W.T, b[None, :]], 0).astype(bf)
    wqT_ext = np.concatenate([Wq.T, bq[None, :]], 0).astype(bf)
    wkT_ext = np.concatenate([Wk.T, bk[None, :]], 0).astype(bf)
    bigI = (np.eye(128, dtype=np.float32) * big).astype(f8)

    in_maps = []
    for c in range(n_cores):
        r0 = c * rr
        slab = adj[r0:r0 + rr]                       # [1536, 12288]
        # adjt[(ch*12+o)*128+p, t*512+q] = slab[ch*512+q, (o*8+t)*128+p]
        # (per-partition 4KB contiguous lines for the octet DMAs)
        a6 = slab.reshape(3, 512, 12, 8, 128).transpose(0, 2, 4, 3, 1)
        adjt = np.ascontiguousarray(a6.reshape(3 * 12 * 128, 8 * 512)).astype(f8)
        in_maps.append({
            "adjt": adjt,
            "xT": np.ascontiguousarray(xT_ext),
            "xqT": np.ascontiguousarray(xT_ext[:, r0:r0 + rr]),
            "wT": wT_ext,
            "wqT": wqT_ext,
            "wkT": wkT_ext,
            "bigi": bigI,
        })
    return in_maps


def consts_from_inputs(inputs):
    scale = float(np.asarray(inputs["scale"], np.float32))
    scale_q = float(np.asarray(inputs["scale_q"], np.float32))
    scale_k = float(np.asarray(inputs["scale_k"], np.float32))
    att_bias = float(np.asarray(inputs["att_bias"], np.float32))
    att_scale = float(np.asarray(inputs["att_scale"], np.float32))
    esc = math.exp(scale)
    esc_q = math.exp(scale_q)
    esc_k = math.exp(scale_k)
    sig_scale = 2.0 / att_scale
    sig_bias = 2.0 / att_scale + att_bias
    return esc, esc_q, esc_k, sig_scale, sig_bias


def kernel(**inputs):
    nn, rr = N_FULL, R_FULL
    consts = consts_from_inputs(inputs)
    nc = build(nn, rr, *consts)
    in_maps = make_in_maps(inputs, nn, rr, N_CORES)
    res = bass_utils.run_bass_kernel_spmd(nc, in_maps,
                                          core_ids=list(range(N_CORES)))
    return np.concatenate([res.results[c]["out"] for c in range(N_CORES)],
                          axis=0)
